# revision 23
# baseline (speedup 1.0000x reference)
"""Multi-head attention block (B=8, N=1024, H=8, d=128, D_in=256) on 8 trn2 cores.

Sharding: data-parallel over batch — core b computes batch element b entirely
(8 heads), no collectives. Host pre-transposes x and pre-scales wq by
1/sqrt(d); the additive [N,N] bias is shipped as exp(B)^T so the device does
exp(S+B) = exp(S) * expB with element-wise engines instead of an
identity-matmul PSUM preload (saves 65k PE rows).

Per-core dataflow (all matmuls float32r, moving free dim 512):
  QT[c,n], KT[c,n] = w.T @ x.T    (c-major so head slices are partition chunks)
  V[n,c]          = x @ wv        (n-major so PV stationary is a natural slice)
  head loop over 16 blocks t=(h,half), software-pipelined at m-granularity:
    S_T[m,n] = KT_h[d,m].T @ QT_h[d,n]     (single matmul per tile)
    at       = exp(S_T) * expB_T[m,n]      (ACT exp; mul split GPSIMD/DVE)
    rs[1,n]  = ones.T @ at                 (softmax denominator, PSUM-accum)
    pv[d,n]  = V_h[m,d].T @ at             (unnormalized, PSUM-accum)
    drain: recip -> DRAM-roundtrip partition-broadcast -> oh = pv * bc
    pj[j,n]  = pw_h[c,j].T @ oh ; yacc += pj
  yT = yacc + proj_b -> DRAM [128, 1024]; host transposes back.

Blocks are half-heads so each drain (DMA-latency-bound) hides behind the next
block's PE work. Emission order per block t: oh-mul(t-2) first (DVE), then the
m-loop [S(t,m) + ones/pv(t-1,m)], then recip/bcast(t-1), then pj/yacc(t-2) —
so the PE never queues behind a DMA-latency-bound op. PSUM: S/pj pool 3,
PV 3, RS 2 banks. QKV-projection setup is woven into blocks 0-7.
"""

import math
import sys

import numpy as np

if "/opt/trn_rl_repo" not in sys.path:
    sys.path.insert(0, "/opt/trn_rl_repo")

import ml_dtypes
import concourse.bass as bass
import concourse.tile as tile
from concourse import bacc
from concourse import mybir

F32 = mybir.dt.float32
F32R = mybir.dt.float32r
BF16 = mybir.dt.bfloat16
EXP = mybir.ActivationFunctionType.Exp
IDENT = mybir.ActivationFunctionType.Identity

N = 1024          # sequence length
D_IN = 256        # input dim
H = 8             # heads
DH = 128          # head dim
C = H * DH        # 1024
NCORES = 8
HALF = 512        # matmul moving free dim
NBLK = 16         # (head, half) blocks
POOL_MULS = 3     # expB muls per block routed to gpsimd (rest on DVE)


def build_nc():
    nc = bacc.Bacc("TRN2", target_bir_lowering=False, debug=False,
                   num_devices=NCORES)

    # 6 input transfers, host-packed so each DMA-completion semaphore
    # carries exactly one transfer (waits are then precise, no aliasing)
    xt_d = nc.dram_tensor("xt", [128, 2 * N], F32R, kind="ExternalInput").ap()
    wqk0_d = nc.dram_tensor("wqk0", [128, 4 * 128], F32R,
                            kind="ExternalInput").ap()
    bias_d = nc.dram_tensor("biases", [128, 1041], F32,
                            kind="ExternalInput").ap()
    wv_d = nc.dram_tensor("wv", [128, 2 * 1024], F32R,
                          kind="ExternalInput").ap()
    wbig_d = nc.dram_tensor("wbig", [128, 2 * 2304], F32R,
                            kind="ExternalInput").ap()
    eb_d = nc.dram_tensor("eb", [128, 8 * N], F32, kind="ExternalInput").ap()
    yT = nc.dram_tensor("yT", [DH, N], F32, kind="ExternalOutput").ap()

    with tile.TileContext(nc) as tc:
        build_body(nc, tc, xt_d, wqk0_d, bias_d, wv_d, wbig_d, eb_d, yT)
    nc.compile()
    return nc


def build_body(nc, tc, xt_d, wqk0_d, bias_d, wv_d, wbig_d, eb_d, yT):
    with (
        tc.tile_pool(name="persist", bufs=1) as P,
        tc.tile_pool(name="at", bufs=9) as AT,
        tc.tile_pool(name="oh", bufs=2) as OH,
        tc.tile_pool(name="rc", bufs=2) as RC,
        tc.tile_pool(name="bcs", bufs=2) as BCS,
        tc.tile_pool(name="ps_s", bufs=3, space="PSUM") as PS_S,
        tc.tile_pool(name="ps_pj", bufs=1, space="PSUM") as PS_PJ,
        tc.tile_pool(name="ps_bc", bufs=1, space="PSUM") as PS_BC,
        tc.tile_pool(name="ps_pv", bufs=2, space="PSUM") as PS_PV,
        tc.tile_pool(name="ps_rs", bufs=1, space="PSUM") as PS_RS,
    ):
        # ---- input DMAs: 8 transfers, critical-path first; each DMA-HW
        # semaphore carries at most one input transfer (waits are precise)
        xt_q = [[P.tile([128, HALF], F32R, tag=f"xt{d}{i}", name=f"xt{d}{i}")
                 for i in range(2)] for d in range(2)]
        for d in range(2):
            for i in range(2):
                nc.sync.dma_start(
                    out=xt_q[d][i],
                    in_=xt_d[:, d * N + i * HALF:d * N + (i + 1) * HALF])
        wqk0 = P.tile([128, 2, 2, 128], F32R, tag="wqk0")
        nc.sync.dma_start(out=wqk0, in_=wqk0_d.rearrange(
            "p (w a c) -> p w a c", w=2, a=2))
        bias_all = P.tile([128, 1041], F32, tag="bias")
        nc.sync.dma_start(out=bias_all, in_=bias_d)
        wv_sb = P.tile([128, 2, 1024], F32R, tag="wv")
        nc.sync.dma_start(out=wv_sb, in_=wv_d.rearrange("p (a c) -> p a c",
                                                        a=2))
        wbig = P.tile([128, 2, 2304], F32R, tag="wbig")
        nc.sync.dma_start(out=wbig, in_=wbig_d.rearrange("p (a c) -> p a c",
                                                         a=2))
        eb_sb = [P.tile([128, 4, N], F32, tag=f"eb{h}", name=f"eb{h}")
                 for h in range(2)]
        for h in range(2):
            nc.sync.dma_start(out=eb_sb[h], in_=eb_d[:, h * 4 * N:(h + 1) * 4 * N]
                              .rearrange("p (a n) -> p a n", a=4))

        def eb_view(m):
            return eb_sb[m // 4][:, m % 4, :]

        wqb_sb = bias_all[:, 0:8]
        wkb_sb = bias_all[:, 8:16]
        wvbb_sb = bias_all[:, 16:1040]
        pb_sb = bias_all[:, 1040:1041]

        def pw_view(h):  # pw head h lives in the d=h//4 tail of wbig
            o = 1792 + (h % 4) * 128
            return wbig[:, h // 4, o:o + 128]

        # ---- persistent tiles ----
        ones = P.tile([128, 1], F32R, tag="ones")
        ones_row = P.tile([1, 128], F32R, tag="ones_row")
        with tc.tile_pool(name="mkconst", bufs=1) as MK:
            ones_f = MK.tile([128, 1], F32, tag="ones_f")
            nc.vector.memset(ones_f, 1.0)
            nc.vector.tensor_copy(ones, ones_f)
            warm = MK.tile([128, 1], F32, tag="warm")
            nc.scalar.activation(warm, ones_f, func=EXP)
            onesr_f = MK.tile([1, 128], F32, tag="onesr_f")
            nc.vector.memset(onesr_f, 1.0)
            nc.vector.tensor_copy(ones_row, onesr_f)
        qt_sb = [P.tile([128, N], F32R, tag=f"qt{c}", name=f"qt{c}") for c in range(8)]
        kt_sb = [P.tile([128, N], F32R, tag=f"kt{c}", name=f"kt{c}") for c in range(8)]
        v_sb = [P.tile([128, C], F32R, tag=f"v{n}", name=f"v{n}") for n in range(8)]
        yacc = P.tile([128, N], F32, tag="yacc")
        yt_sb = P.tile([128, N], F32, tag="yt")

        # ---- setup pieces (emitted interleaved into early blocks) ----
        def qkt_piece(wname, b_sb, dst, c, on_act):
            wi = 0 if wname == "wq" else 1
            for i in range(2):
                ns = slice(i * HALF, (i + 1) * HALF)
                ps = PS_S.tile([128, HALF], F32)
                for d in range(2):
                    if c == 0:
                        wt = wqk0[:, wi, d, :]
                    else:
                        wt = wbig[:, d, wi * 896 + (c - 1) * 128:
                                  wi * 896 + c * 128]
                    nc.tensor.matmul(ps, wt, xt_q[d][i],
                                     start=(d == 0), stop=(d == 1))
                if on_act:
                    nc.scalar.activation(dst[c][:, ns], ps, func=IDENT,
                                         bias=b_sb[:, c:c + 1])
                else:
                    nc.vector.tensor_scalar_add(dst[c][:, ns], ps,
                                                b_sb[:, c:c + 1])

        def v_piece(n):
            nsl = slice(n * 128, (n + 1) * 128)
            for i in range(2):
                cs = slice(i * HALF, (i + 1) * HALF)
                ps = PS_S.tile([128, HALF], F32)
                nc.tensor.matmul(ps, xt_q[0][n // 4][:, (n % 4) * 128:
                                                         (n % 4 + 1) * 128],
                                 wv_sb[:, 0, cs], start=True, stop=False)
                nc.tensor.matmul(ps, xt_q[1][n // 4][:, (n % 4) * 128:
                                                         (n % 4 + 1) * 128],
                                 wv_sb[:, 1, cs], start=False, stop=True)
                nc.vector.tensor_add(v_sb[n][:, cs], ps, wvbb_sb[:, cs])

        # qt/kt c0 first so block 0's S matmuls can start immediately
        qkt_piece("wq", wqb_sb, qt_sb, 0, True)
        qkt_piece("wk", wkb_sb, kt_sb, 0, False)

        # remaining pieces woven into blocks: V into block 0 (needed by the
        # first ones/pv in block 1), qt/kt chunk c before block 2c
        pieces = [lambda n=n: v_piece(n) for n in range(8)]
        for c in range(1, 8):
            pieces.append(lambda c=c: qkt_piece("wq", wqb_sb, qt_sb, c, True))
            pieces.append(lambda c=c: qkt_piece("wk", wkb_sb, kt_sb, c, False))
        piece_quota = {0: 8, 1: 2, 2: 2, 3: 2, 4: 2, 5: 2, 6: 2, 7: 2}

        # ---- pipelined block loop: block t = (head h, n-half i) ----
        at_t = {}     # (t, m) -> at tile
        pv_t = {}     # t -> pv psum tile
        rs_t = {}     # t -> rowsum psum tile
        rc_t = {}     # t -> reciprocal rowsum [1, HALF]
        bcp_t = {}    # t -> PE-broadcast recip psum tile
        oh_t = {}     # t -> normalized head-output tile

        def s_exp_mul(t, m):
            h, i = divmod(t, 2)
            ns = slice(i * HALF, (i + 1) * HALF)
            ms = slice(m * 128, (m + 1) * 128)
            ps = PS_S.tile([128, HALF], F32)
            nc.tensor.matmul(ps, kt_sb[h][:, ms], qt_sb[h][:, ns],
                             start=True, stop=True)
            at = AT.tile([128, HALF], F32R)
            nc.scalar.activation(at, ps, func=EXP)
            eng = nc.gpsimd if (m < POOL_MULS or t == 0) else nc.vector
            eng.tensor_mul(at, at, eb_view(m)[:, ns])
            at_t[(t, m)] = at

        def ones_pv(t, m):
            h, _ = divmod(t, 2)
            hs = slice(h * 128, (h + 1) * 128)
            if m == 0:
                rs_t[t] = PS_RS.tile([1, HALF], F32, tag="rs", name=f"rs{t}")
                pv_t[t] = PS_PV.tile([128, HALF], F32, tag="pv", name=f"pv{t}")
            at = at_t.pop((t, m))
            nc.tensor.matmul(rs_t[t], ones, at, start=(m == 0), stop=(m == 7))
            nc.tensor.matmul(pv_t[t], v_sb[m][:, hs], at,
                             start=(m == 0), stop=(m == 7))

        from concourse.dve_ops import (
            RECIP_APPROX_FAST_CONSTS,
            RECIPROCAL_APPROX_FAST,
        )

        def recip(t):
            # softmax denominators: 1/rowsum, approx (~18 good bits), f32r
            # out so the broadcast matmul can consume it directly
            rc = RC.tile([1, HALF], F32R, tag="rc", name=f"rc{t}")
            cc = RECIP_APPROX_FAST_CONSTS
            nc.vector._custom_dve(RECIPROCAL_APPROX_FAST, out=rc,
                                  in0=rs_t.pop(t), s0=cc["s0"], s1=cc["s1"],
                                  imm2=cc["imm2"])
            rc_t[t] = rc

        def bcp_mm(t):
            # partition-broadcast recip via contraction-1 matmul (no DMA)
            bcp = PS_BC.tile([128, HALF], F32, tag="bcp", name=f"bcp{t}")
            nc.tensor.matmul(bcp, ones_row, rc_t.pop(t),
                             start=True, stop=True)
            bcs = BCS.tile([128, HALF], F32, tag="bcs", name=f"bcs{t}")
            nc.scalar.activation(bcs, bcp, func=IDENT)
            bcp_t[t] = bcs

        def oh_mul(t):
            oh = OH.tile([128, HALF], F32R, tag="oh", name=f"oh{t}")
            nc.vector.tensor_mul(oh, pv_t.pop(t), bcp_t.pop(t))
            oh_t[t] = oh

        def proj_acc(t):
            h, i = divmod(t, 2)
            ns = slice(i * HALF, (i + 1) * HALF)
            pj = PS_PJ.tile([128, HALF], F32, tag="pj", name=f"pj{t}")
            nc.tensor.matmul(pj, pw_view(h), oh_t.pop(t),
                             start=True, stop=True)
            if h == 0:
                nc.vector.tensor_copy(yacc[:, ns], pj)
            else:
                nc.vector.tensor_add(yacc[:, ns], yacc[:, ns], pj)

        def finalize(i):
            ns = slice(i * HALF, (i + 1) * HALF)
            nc.scalar.activation(yt_sb[:, ns], yacc[:, ns], func=IDENT,
                                 bias=pb_sb)
            nc.sync.dma_start(out=yT[:, ns], in_=yt_sb[:, ns])

        pi = 0
        for t in range(NBLK + 2):
            quota = piece_quota.get(t, 0)
            for m in range(8):
                if t < NBLK:
                    s_exp_mul(t, m)
                if 1 <= t <= NBLK:
                    ones_pv(t - 1, m)
                if m == 2 and 2 <= t:
                    bcp_mm(t - 2)    # recip(t-2) done by now; 213ns on PE
                if m == 4 and 2 <= t:
                    oh_mul(t - 2)    # bcp just above; frees pv(t-2)
                if quota and m % (8 // quota) == (8 // quota) - 1:
                    pieces[pi](); pi += 1
            if 1 <= t <= NBLK:
                recip(t - 1)         # rs(t-1) just stopped
            if 2 <= t:
                proj_acc(t - 2)      # PE reaches this after the block's work
                if t - 2 >= NBLK - 2:
                    finalize((t - 2) % 2)
        assert pi == len(pieces)


_CACHE = {}


def _prep_inputs(x, B_bias, wq_w, wq_b, wk_w, wk_b, wv_w, wv_b, proj_w, proj_b):
    s = 1.0 / math.sqrt(DH)
    f = np.float32

    def d2(w):  # [256, C] -> [2, 128, C]
        return np.asarray(w, f).reshape(2, 128, -1)

    wq3 = d2(np.asarray(wq_w) * s)
    wk3 = d2(wk_w)
    wv3 = d2(wv_w)
    # wqk0: [p, w, a, c0] packed
    wqk0 = np.stack([wq3[:, :, :128], wk3[:, :, :128]], 0)  # [w, a, p, 128]
    wqk0 = np.ascontiguousarray(wqk0.transpose(2, 0, 1, 3).reshape(128, -1))
    # wbig per d: [wq_r 896 | wk_r 896 | wv 1024 | pw-half 512]
    pwf = np.asarray(proj_w, f).reshape(2, 512, DH)  # head-halves 0-3 / 4-7
    rows = []
    for d in range(2):
        pw_tail = pwf[d].reshape(4, 128, DH)
        pw_part = pw_tail.transpose(1, 0, 2).reshape(128, 512)
        rows.append(np.concatenate(
            [wq3[d, :, 128:], wk3[d, :, 128:], pw_part], axis=1))
    wbig = np.ascontiguousarray(np.stack(rows, 1).reshape(128, -1))
    wv_p = np.ascontiguousarray(wv3.transpose(1, 0, 2).reshape(128, -1))
    # biases: [wqb 8 | wkb 8 | wvbb 1024 | pb 1]
    wqb_t = (np.asarray(wq_b, f) * s).reshape(8, 128).T
    wkb_t = np.asarray(wk_b, f).reshape(8, 128).T
    wvbb = np.broadcast_to(np.asarray(wv_b, f), (128, C))
    pb_t = np.asarray(proj_b, f).reshape(128, 1)
    bias_all = np.ascontiguousarray(
        np.concatenate([wqb_t, wkb_t, wvbb, pb_t], axis=1))
    # eb: exp(B)^T chunks packed [p, (m n)]
    ebh = np.exp(np.asarray(B_bias, np.float32).T).reshape(8, 128, N)
    eb_all = np.ascontiguousarray(
        ebh.transpose(1, 0, 2).reshape(128, 8 * N))
    xTh = np.asarray(x, f).transpose(0, 2, 1).reshape(8, 2, 128, N)
    shared = dict(wqk0=wqk0, wbig=wbig, wv=wv_p, biases=bias_all, eb=eb_all)
    return [dict(shared, xt=np.ascontiguousarray(
        xTh[b].transpose(1, 0, 2).reshape(128, 2 * N))) for b in range(NCORES)]


def kernel(**inputs):
    from concourse.bass_utils import run_bass_kernel_spmd

    if "nc" not in _CACHE:
        _CACHE["nc"] = build_nc()
    nc = _CACHE["nc"]
    in_maps = _prep_inputs(**inputs)
    res = run_bass_kernel_spmd(nc, in_maps, core_ids=list(range(NCORES)))
    out = np.stack([np.asarray(res.results[b]["yT"]).T for b in range(NCORES)])
    return np.ascontiguousarray(out.astype(np.float32))


# revision 24
# speedup vs baseline: 1.1741x; 1.1741x over previous
"""Multi-head attention block (B=8, N=1024, H=8, d=128, D_in=256) on 8 trn2 cores.

Sharding: data-parallel over batch — core b computes batch element b entirely
(8 heads), no collectives. Host pre-transposes x and pre-scales wq by
1/sqrt(d); the additive [N,N] bias is shipped as exp(B)^T so the device does
exp(S+B) = exp(S) * expB with element-wise engines instead of an
identity-matmul PSUM preload (saves 65k PE rows).

Per-core dataflow (all matmuls float32r, moving free dim 512):
  QT[c,n], KT[c,n] = w.T @ x.T    (c-major so head slices are partition chunks)
  V[n,c]          = x @ wv        (n-major so PV stationary is a natural slice)
  head loop over 16 blocks t=(h,half), software-pipelined at m-granularity:
    S_T[m,n] = KT_h[d,m].T @ QT_h[d,n]     (single matmul per tile)
    at       = exp(S_T) * expB_T[m,n]      (ACT exp; mul split GPSIMD/DVE)
    rs[1,n]  = ones.T @ at                 (softmax denominator, PSUM-accum)
    pv[d,n]  = V_h[m,d].T @ at             (unnormalized, PSUM-accum)
    drain: recip -> DRAM-roundtrip partition-broadcast -> oh = pv * bc
    pj[j,n]  = pw_h[c,j].T @ oh ; yacc += pj
  yT = yacc + proj_b -> DRAM [128, 1024]; host transposes back.

Blocks are half-heads so each drain (DMA-latency-bound) hides behind the next
block's PE work. Emission order per block t: oh-mul(t-2) first (DVE), then the
m-loop [S(t,m) + ones/pv(t-1,m)], then recip/bcast(t-1), then pj/yacc(t-2) —
so the PE never queues behind a DMA-latency-bound op. PSUM: S/pj pool 3,
PV 3, RS 2 banks. QKV-projection setup is woven into blocks 0-7.
"""

import math
import sys

import numpy as np

if "/opt/trn_rl_repo" not in sys.path:
    sys.path.insert(0, "/opt/trn_rl_repo")

import ml_dtypes
import concourse.bass as bass
import concourse.tile as tile
from concourse import bacc
from concourse import mybir

F32 = mybir.dt.float32
F32R = mybir.dt.float32r
BF16 = mybir.dt.bfloat16
EXP = mybir.ActivationFunctionType.Exp
IDENT = mybir.ActivationFunctionType.Identity

N = 1024          # sequence length
D_IN = 256        # input dim
H = 8             # heads
DH = 128          # head dim
C = H * DH        # 1024
NCORES = 8
HALF = 512        # matmul moving free dim
NBLK = 16         # (head, half) blocks
POOL_MULS = 3     # expB muls per block routed to gpsimd (rest on DVE)


def build_nc():
    nc = bacc.Bacc("TRN2", target_bir_lowering=False, debug=False,
                   num_devices=NCORES)

    # 6 input transfers, host-packed so each DMA-completion semaphore
    # carries exactly one transfer (waits are then precise, no aliasing)
    xt_d = nc.dram_tensor("xt", [128, 2 * N], F32R, kind="ExternalInput").ap()
    wqk0_d = nc.dram_tensor("wqk0", [128, 4 * 128], F32R,
                            kind="ExternalInput").ap()
    bias_d = nc.dram_tensor("biases", [128, 1041], F32,
                            kind="ExternalInput").ap()
    wv_d = nc.dram_tensor("wv", [128, 2 * 1024], F32R,
                          kind="ExternalInput").ap()
    wbig_d = nc.dram_tensor("wbig", [128, 2 * 2304], F32R,
                            kind="ExternalInput").ap()
    eb_d = nc.dram_tensor("eb", [128, 8 * N], F32, kind="ExternalInput").ap()
    yT = nc.dram_tensor("yT", [DH, N], F32, kind="ExternalOutput").ap()

    with tile.TileContext(nc) as tc:
        build_body(nc, tc, xt_d, wqk0_d, bias_d, wv_d, wbig_d, eb_d, yT)
    nc.compile()
    return nc


def build_body(nc, tc, xt_d, wqk0_d, bias_d, wv_d, wbig_d, eb_d, yT):
    with (
        tc.tile_pool(name="persist", bufs=1) as P,
        tc.tile_pool(name="at", bufs=9) as AT,
        tc.tile_pool(name="oh", bufs=2) as OH,
        tc.tile_pool(name="rc", bufs=2) as RC,
        tc.tile_pool(name="bcs", bufs=2) as BCS,
        tc.tile_pool(name="ps_s", bufs=3, space="PSUM") as PS_S,
        tc.tile_pool(name="ps_pj", bufs=1, space="PSUM") as PS_PJ,
        tc.tile_pool(name="ps_bc", bufs=1, space="PSUM") as PS_BC,
        tc.tile_pool(name="ps_pv", bufs=2, space="PSUM") as PS_PV,
        tc.tile_pool(name="ps_rs", bufs=1, space="PSUM") as PS_RS,
    ):
        # ---- input DMAs: 8 transfers, critical-path first; each DMA-HW
        # semaphore carries at most one input transfer (waits are precise)
        xt_all = [P.tile([128, N], F32R, tag=f"xt{d}", name=f"xt{d}")
                  for d in range(2)]
        for d in range(2):
            nc.sync.dma_start(out=xt_all[d], in_=xt_d[:, d * N:(d + 1) * N])
        wqk0 = P.tile([128, 2, 2, 128], F32R, tag="wqk0")
        nc.sync.dma_start(out=wqk0, in_=wqk0_d.rearrange(
            "p (w a c) -> p w a c", w=2, a=2))
        bias_all = P.tile([128, 1041], F32, tag="bias")
        nc.sync.dma_start(out=bias_all, in_=bias_d)
        wv_sb = P.tile([128, 2, 1024], F32R, tag="wv")
        nc.sync.dma_start(out=wv_sb, in_=wv_d.rearrange("p (a c) -> p a c",
                                                        a=2))
        wbig = P.tile([128, 2, 2304], F32R, tag="wbig")
        nc.sync.dma_start(out=wbig, in_=wbig_d.rearrange("p (a c) -> p a c",
                                                         a=2))
        eb_sb = [P.tile([128, 4, N], F32, tag=f"eb{h}", name=f"eb{h}")
                 for h in range(2)]
        for h in range(2):
            nc.sync.dma_start(out=eb_sb[h], in_=eb_d[:, h * 4 * N:(h + 1) * 4 * N]
                              .rearrange("p (a n) -> p a n", a=4))

        def eb_view(m):
            return eb_sb[m // 4][:, m % 4, :]

        wqb_sb = bias_all[:, 0:8]
        wkb_sb = bias_all[:, 8:16]
        wvbb_sb = bias_all[:, 16:1040]
        pb_sb = bias_all[:, 1040:1041]

        def pw_view(h):  # pw head h lives in the d=h//4 tail of wbig
            o = 1792 + (h % 4) * 128
            return wbig[:, h // 4, o:o + 128]

        # ---- persistent tiles ----
        ones = P.tile([128, 1], F32R, tag="ones")
        ones_row = P.tile([1, 128], F32R, tag="ones_row")
        with tc.tile_pool(name="mkconst", bufs=1) as MK:
            ones_f = MK.tile([128, 1], F32, tag="ones_f")
            nc.vector.memset(ones_f, 1.0)
            nc.vector.tensor_copy(ones, ones_f)
            warm = MK.tile([128, 1], F32, tag="warm")
            nc.scalar.activation(warm, ones_f, func=EXP)
            onesr_f = MK.tile([1, 128], F32, tag="onesr_f")
            nc.vector.memset(onesr_f, 1.0)
            nc.vector.tensor_copy(ones_row, onesr_f)
        qt_sb = [P.tile([128, N], F32R, tag=f"qt{c}", name=f"qt{c}") for c in range(8)]
        kt_sb = [P.tile([128, N], F32R, tag=f"kt{c}", name=f"kt{c}") for c in range(8)]
        v_sb = [P.tile([128, C], F32R, tag=f"v{n}", name=f"v{n}") for n in range(8)]
        yacc = P.tile([128, N], F32, tag="yacc")
        yt_sb = P.tile([128, N], F32, tag="yt")

        # ---- setup pieces (emitted interleaved into early blocks) ----
        def qkt_piece(wname, b_sb, dst, c, on_act):
            wi = 0 if wname == "wq" else 1
            for i in range(2):
                ns = slice(i * HALF, (i + 1) * HALF)
                ps = PS_S.tile([128, HALF], F32)
                for d in range(2):
                    if c == 0:
                        wt = wqk0[:, wi, d, :]
                    else:
                        wt = wbig[:, d, wi * 896 + (c - 1) * 128:
                                  wi * 896 + c * 128]
                    nc.tensor.matmul(ps, wt, xt_all[d][:, ns],
                                     start=(d == 0), stop=(d == 1))
                if on_act:
                    nc.scalar.activation(dst[c][:, ns], ps, func=IDENT,
                                         bias=b_sb[:, c:c + 1])
                else:
                    nc.vector.tensor_scalar_add(dst[c][:, ns], ps,
                                                b_sb[:, c:c + 1])

        def v_piece(n):
            nsl = slice(n * 128, (n + 1) * 128)
            for i in range(2):
                cs = slice(i * HALF, (i + 1) * HALF)
                ps = PS_S.tile([128, HALF], F32)
                nc.tensor.matmul(ps, xt_all[0][:, nsl], wv_sb[:, 0, cs],
                                 start=True, stop=False)
                nc.tensor.matmul(ps, xt_all[1][:, nsl], wv_sb[:, 1, cs],
                                 start=False, stop=True)
                nc.vector.tensor_add(v_sb[n][:, cs], ps, wvbb_sb[:, cs])

        # qt/kt c0 first so block 0's S matmuls can start immediately
        qkt_piece("wq", wqb_sb, qt_sb, 0, True)
        qkt_piece("wk", wkb_sb, kt_sb, 0, False)

        # remaining pieces woven into blocks: V into block 0 (needed by the
        # first ones/pv in block 1), qt/kt chunk c before block 2c
        pieces = [lambda n=n: v_piece(n) for n in range(8)]
        for c in range(1, 8):
            pieces.append(lambda c=c: qkt_piece("wq", wqb_sb, qt_sb, c, True))
            pieces.append(lambda c=c: qkt_piece("wk", wkb_sb, kt_sb, c, False))
        piece_quota = {0: 8, 1: 2, 2: 2, 3: 2, 4: 2, 5: 2, 6: 2, 7: 2}

        # ---- pipelined block loop: block t = (head h, n-half i) ----
        at_t = {}     # (t, m) -> at tile
        pv_t = {}     # t -> pv psum tile
        rs_t = {}     # t -> rowsum psum tile
        rc_t = {}     # t -> reciprocal rowsum [1, HALF]
        bcp_t = {}    # t -> PE-broadcast recip psum tile
        oh_t = {}     # t -> normalized head-output tile

        def s_exp_mul(t, m):
            h, i = divmod(t, 2)
            ns = slice(i * HALF, (i + 1) * HALF)
            ms = slice(m * 128, (m + 1) * 128)
            ps = PS_S.tile([128, HALF], F32)
            nc.tensor.matmul(ps, kt_sb[h][:, ms], qt_sb[h][:, ns],
                             start=True, stop=True)
            at = AT.tile([128, HALF], F32R)
            nc.scalar.activation(at, ps, func=EXP)
            eng = nc.gpsimd if (m < POOL_MULS or t == 0) else nc.vector
            eng.tensor_mul(at, at, eb_view(m)[:, ns])
            at_t[(t, m)] = at

        def ones_pv(t, m):
            h, _ = divmod(t, 2)
            hs = slice(h * 128, (h + 1) * 128)
            if m == 0:
                rs_t[t] = PS_RS.tile([1, HALF], F32, tag="rs", name=f"rs{t}")
                pv_t[t] = PS_PV.tile([128, HALF], F32, tag="pv", name=f"pv{t}")
            at = at_t.pop((t, m))
            nc.tensor.matmul(rs_t[t], ones, at, start=(m == 0), stop=(m == 7))
            nc.tensor.matmul(pv_t[t], v_sb[m][:, hs], at,
                             start=(m == 0), stop=(m == 7))

        from concourse.dve_ops import (
            RECIP_APPROX_FAST_CONSTS,
            RECIPROCAL_APPROX_FAST,
        )

        def recip(t):
            # softmax denominators: 1/rowsum, approx (~18 good bits), f32r
            # out so the broadcast matmul can consume it directly
            rc = RC.tile([1, HALF], F32R, tag="rc", name=f"rc{t}")
            cc = RECIP_APPROX_FAST_CONSTS
            nc.vector._custom_dve(RECIPROCAL_APPROX_FAST, out=rc,
                                  in0=rs_t.pop(t), s0=cc["s0"], s1=cc["s1"],
                                  imm2=cc["imm2"])
            rc_t[t] = rc

        def bcp_mm(t):
            # partition-broadcast recip via contraction-1 matmul (no DMA)
            bcp = PS_BC.tile([128, HALF], F32, tag="bcp", name=f"bcp{t}")
            nc.tensor.matmul(bcp, ones_row, rc_t.pop(t),
                             start=True, stop=True)
            bcs = BCS.tile([128, HALF], F32, tag="bcs", name=f"bcs{t}")
            nc.scalar.activation(bcs, bcp, func=IDENT)
            bcp_t[t] = bcs

        def oh_mul(t):
            oh = OH.tile([128, HALF], F32R, tag="oh", name=f"oh{t}")
            nc.vector.tensor_mul(oh, pv_t.pop(t), bcp_t.pop(t))
            oh_t[t] = oh

        def proj_acc(t):
            h, i = divmod(t, 2)
            ns = slice(i * HALF, (i + 1) * HALF)
            pj = PS_PJ.tile([128, HALF], F32, tag="pj", name=f"pj{t}")
            nc.tensor.matmul(pj, pw_view(h), oh_t.pop(t),
                             start=True, stop=True)
            if h == 0:
                nc.vector.tensor_copy(yacc[:, ns], pj)
            else:
                nc.vector.tensor_add(yacc[:, ns], yacc[:, ns], pj)

        def finalize(i):
            ns = slice(i * HALF, (i + 1) * HALF)
            nc.scalar.activation(yt_sb[:, ns], yacc[:, ns], func=IDENT,
                                 bias=pb_sb)
            nc.sync.dma_start(out=yT[:, ns], in_=yt_sb[:, ns])

        pi = 0
        for t in range(NBLK + 2):
            quota = piece_quota.get(t, 0)
            for m in range(8):
                if t < NBLK:
                    s_exp_mul(t, m)
                if 1 <= t <= NBLK:
                    ones_pv(t - 1, m)
                if m == 2 and 2 <= t:
                    bcp_mm(t - 2)    # recip(t-2) done by now; 213ns on PE
                if m == 4 and 2 <= t:
                    oh_mul(t - 2)    # bcp just above; frees pv(t-2)
                if quota and m % (8 // quota) == (8 // quota) - 1:
                    pieces[pi](); pi += 1
            if 1 <= t <= NBLK:
                recip(t - 1)         # rs(t-1) just stopped
            if 2 <= t:
                proj_acc(t - 2)      # PE reaches this after the block's work
                if t - 2 >= NBLK - 2:
                    finalize((t - 2) % 2)
        assert pi == len(pieces)


_CACHE = {}


def _prep_inputs(x, B_bias, wq_w, wq_b, wk_w, wk_b, wv_w, wv_b, proj_w, proj_b):
    s = 1.0 / math.sqrt(DH)
    f = np.float32

    def d2(w):  # [256, C] -> [2, 128, C]
        return np.asarray(w, f).reshape(2, 128, -1)

    wq3 = d2(np.asarray(wq_w) * s)
    wk3 = d2(wk_w)
    wv3 = d2(wv_w)
    # wqk0: [p, w, a, c0] packed
    wqk0 = np.stack([wq3[:, :, :128], wk3[:, :, :128]], 0)  # [w, a, p, 128]
    wqk0 = np.ascontiguousarray(wqk0.transpose(2, 0, 1, 3).reshape(128, -1))
    # wbig per d: [wq_r 896 | wk_r 896 | wv 1024 | pw-half 512]
    pwf = np.asarray(proj_w, f).reshape(2, 512, DH)  # head-halves 0-3 / 4-7
    rows = []
    for d in range(2):
        pw_tail = pwf[d].reshape(4, 128, DH)
        pw_part = pw_tail.transpose(1, 0, 2).reshape(128, 512)
        rows.append(np.concatenate(
            [wq3[d, :, 128:], wk3[d, :, 128:], pw_part], axis=1))
    wbig = np.ascontiguousarray(np.stack(rows, 1).reshape(128, -1))
    wv_p = np.ascontiguousarray(wv3.transpose(1, 0, 2).reshape(128, -1))
    # biases: [wqb 8 | wkb 8 | wvbb 1024 | pb 1]
    wqb_t = (np.asarray(wq_b, f) * s).reshape(8, 128).T
    wkb_t = np.asarray(wk_b, f).reshape(8, 128).T
    wvbb = np.broadcast_to(np.asarray(wv_b, f), (128, C))
    pb_t = np.asarray(proj_b, f).reshape(128, 1)
    bias_all = np.ascontiguousarray(
        np.concatenate([wqb_t, wkb_t, wvbb, pb_t], axis=1))
    # eb: exp(B)^T chunks packed [p, (m n)]
    ebh = np.exp(np.asarray(B_bias, np.float32).T).reshape(8, 128, N)
    eb_all = np.ascontiguousarray(
        ebh.transpose(1, 0, 2).reshape(128, 8 * N))
    xTh = np.asarray(x, f).transpose(0, 2, 1).reshape(8, 2, 128, N)
    shared = dict(wqk0=wqk0, wbig=wbig, wv=wv_p, biases=bias_all, eb=eb_all)
    return [dict(shared, xt=np.ascontiguousarray(
        xTh[b].transpose(1, 0, 2).reshape(128, 2 * N))) for b in range(NCORES)]


def kernel(**inputs):
    from concourse.bass_utils import run_bass_kernel_spmd

    if "nc" not in _CACHE:
        _CACHE["nc"] = build_nc()
    nc = _CACHE["nc"]
    in_maps = _prep_inputs(**inputs)
    res = run_bass_kernel_spmd(nc, in_maps, core_ids=list(range(NCORES)))
    out = np.stack([np.asarray(res.results[b]["yT"]).T for b in range(NCORES)])
    return np.ascontiguousarray(out.astype(np.float32))


# revision 26
# speedup vs baseline: 1.1971x; 1.0196x over previous
"""Multi-head attention block (B=8, N=1024, H=8, d=128, D_in=256) on 8 trn2 cores.

Sharding: data-parallel over batch — core b computes batch element b entirely
(8 heads), no collectives. Host pre-transposes x and pre-scales wq by
1/sqrt(d); the additive [N,N] bias is shipped as exp(B)^T so the device does
exp(S+B) = exp(S) * expB with element-wise engines instead of an
identity-matmul PSUM preload (saves 65k PE rows).

Per-core dataflow (all matmuls float32r, moving free dim 512):
  QT[c,n], KT[c,n] = w.T @ x.T    (c-major so head slices are partition chunks)
  V[n,c]          = x @ wv        (n-major so PV stationary is a natural slice)
  head loop over 16 blocks t=(h,half), software-pipelined at m-granularity:
    S_T[m,n] = KT_h[d,m].T @ QT_h[d,n]     (single matmul per tile)
    at       = exp(S_T) * expB_T[m,n]      (ACT exp; mul split GPSIMD/DVE)
    rs[1,n]  = ones.T @ at                 (softmax denominator, PSUM-accum)
    pv[d,n]  = V_h[m,d].T @ at             (unnormalized, PSUM-accum)
    drain: recip -> DRAM-roundtrip partition-broadcast -> oh = pv * bc
    pj[j,n]  = pw_h[c,j].T @ oh ; yacc += pj
  yT = yacc + proj_b -> DRAM [128, 1024]; host transposes back.

Blocks are half-heads so each drain (DMA-latency-bound) hides behind the next
block's PE work. Emission order per block t: oh-mul(t-2) first (DVE), then the
m-loop [S(t,m) + ones/pv(t-1,m)], then recip/bcast(t-1), then pj/yacc(t-2) —
so the PE never queues behind a DMA-latency-bound op. PSUM: S/pj pool 3,
PV 3, RS 2 banks. QKV-projection setup is woven into blocks 0-7.
"""

import math
import sys

import numpy as np

if "/opt/trn_rl_repo" not in sys.path:
    sys.path.insert(0, "/opt/trn_rl_repo")

import ml_dtypes
import concourse.bass as bass
import concourse.tile as tile
from concourse import bacc
from concourse import mybir

F32 = mybir.dt.float32
F32R = mybir.dt.float32r
BF16 = mybir.dt.bfloat16
EXP = mybir.ActivationFunctionType.Exp
IDENT = mybir.ActivationFunctionType.Identity

N = 1024          # sequence length
D_IN = 256        # input dim
H = 8             # heads
DH = 128          # head dim
C = H * DH        # 1024
NCORES = 8
HALF = 512        # matmul moving free dim
NBLK = 16         # (head, half) blocks
POOL_MULS = 3     # expB muls per block routed to gpsimd (rest on DVE)


def build_nc():
    nc = bacc.Bacc("TRN2", target_bir_lowering=False, debug=False,
                   num_devices=NCORES)

    # 6 input transfers, host-packed so each DMA-completion semaphore
    # carries exactly one transfer (waits are then precise, no aliasing)
    xt_d = nc.dram_tensor("xt", [128, 2 * N], F32R, kind="ExternalInput").ap()
    wqk0_d = nc.dram_tensor("wqk0", [128, 4 * 128], F32R,
                            kind="ExternalInput").ap()
    bias_d = nc.dram_tensor("biases", [128, 1041], F32,
                            kind="ExternalInput").ap()
    wv_d = nc.dram_tensor("wv", [128, 2 * 1024], F32R,
                          kind="ExternalInput").ap()
    wbig_d = nc.dram_tensor("wbig", [128, 2 * 2304], F32R,
                            kind="ExternalInput").ap()
    eb_d = nc.dram_tensor("eb", [128, 8 * N], F32, kind="ExternalInput").ap()
    yT = nc.dram_tensor("yT", [DH, N], F32, kind="ExternalOutput").ap()

    with tile.TileContext(nc) as tc:
        build_body(nc, tc, xt_d, wqk0_d, bias_d, wv_d, wbig_d, eb_d, yT)
    nc.compile()
    return nc


def build_body(nc, tc, xt_d, wqk0_d, bias_d, wv_d, wbig_d, eb_d, yT):
    with (
        tc.tile_pool(name="persist", bufs=1) as P,
        tc.tile_pool(name="at", bufs=9) as AT,
        tc.tile_pool(name="oh", bufs=2) as OH,
        tc.tile_pool(name="rc", bufs=2) as RC,
        tc.tile_pool(name="bcs", bufs=2) as BCS,
        tc.tile_pool(name="ps_s", bufs=3, space="PSUM") as PS_S,
        tc.tile_pool(name="ps_pj", bufs=1, space="PSUM") as PS_PJ,
        tc.tile_pool(name="ps_bc", bufs=1, space="PSUM") as PS_BC,
        tc.tile_pool(name="ps_pv", bufs=2, space="PSUM") as PS_PV,
        tc.tile_pool(name="ps_rs", bufs=1, space="PSUM") as PS_RS,
    ):
        # ---- input DMAs: 8 transfers, critical-path first; each DMA-HW
        # semaphore carries at most one input transfer (waits are precise)
        xt_all = [P.tile([128, N], F32R, tag=f"xt{d}", name=f"xt{d}")
                  for d in range(2)]
        for d in range(2):
            nc.sync.dma_start(out=xt_all[d], in_=xt_d[:, d * N:(d + 1) * N])
        wqk0 = P.tile([128, 2, 2, 128], F32R, tag="wqk0")
        nc.sync.dma_start(out=wqk0, in_=wqk0_d.rearrange(
            "p (w a c) -> p w a c", w=2, a=2))
        bias_all = P.tile([128, 1041], F32, tag="bias")
        nc.sync.dma_start(out=bias_all, in_=bias_d)
        wv_sb = P.tile([128, 2, 1024], F32R, tag="wv")
        nc.sync.dma_start(out=wv_sb, in_=wv_d.rearrange("p (a c) -> p a c",
                                                        a=2))
        wbig = P.tile([128, 2, 2304], F32R, tag="wbig")
        nc.sync.dma_start(out=wbig, in_=wbig_d.rearrange("p (a c) -> p a c",
                                                         a=2))
        eb_sb = [P.tile([128, 4, N], F32, tag=f"eb{h}", name=f"eb{h}")
                 for h in range(2)]
        for h in range(2):
            nc.sync.dma_start(out=eb_sb[h], in_=eb_d[:, h * 4 * N:(h + 1) * 4 * N]
                              .rearrange("p (a n) -> p a n", a=4))

        def eb_view(m):
            return eb_sb[m // 4][:, m % 4, :]

        wqb_sb = bias_all[:, 0:8]
        wkb_sb = bias_all[:, 8:16]
        wvbb_sb = bias_all[:, 16:1040]
        pb_sb = bias_all[:, 1040:1041]

        def pw_view(h):  # pw head h lives in the d=h//4 tail of wbig
            o = 1792 + (h % 4) * 128
            return wbig[:, h // 4, o:o + 128]

        # ---- persistent tiles ----
        ones = P.tile([128, 1], F32R, tag="ones")
        ones_row = P.tile([1, 128], F32R, tag="ones_row")
        with tc.tile_pool(name="mkconst", bufs=1) as MK:
            ones_f = MK.tile([128, 1], F32, tag="ones_f")
            nc.vector.memset(ones_f, 1.0)
            nc.vector.tensor_copy(ones, ones_f)
            warm = MK.tile([128, 1], F32, tag="warm")
            nc.scalar.activation(warm, ones_f, func=EXP)
            onesr_f = MK.tile([1, 128], F32, tag="onesr_f")
            nc.vector.memset(onesr_f, 1.0)
            nc.vector.tensor_copy(ones_row, onesr_f)
        qt_sb = [P.tile([128, N], F32R, tag=f"qt{c}", name=f"qt{c}") for c in range(8)]
        kt_sb = [P.tile([128, N], F32R, tag=f"kt{c}", name=f"kt{c}") for c in range(8)]
        v_sb = [P.tile([128, C], F32R, tag=f"v{n}", name=f"v{n}") for n in range(8)]
        yacc = P.tile([128, N], F32, tag="yacc")
        yt_sb = P.tile([128, N], F32, tag="yt")

        # ---- setup pieces (emitted interleaved into early blocks) ----
        def qkt_piece(wname, b_sb, dst, c, on_act):
            wi = 0 if wname == "wq" else 1
            for i in range(2):
                ns = slice(i * HALF, (i + 1) * HALF)
                ps = PS_S.tile([128, HALF], F32)
                for d in range(2):
                    if c == 0:
                        wt = wqk0[:, wi, d, :]
                    else:
                        wt = wbig[:, d, wi * 896 + (c - 1) * 128:
                                  wi * 896 + c * 128]
                    nc.tensor.matmul(ps, wt, xt_all[d][:, ns],
                                     start=(d == 0), stop=(d == 1))
                if on_act:
                    nc.scalar.activation(dst[c][:, ns], ps, func=IDENT,
                                         bias=b_sb[:, c:c + 1])
                else:
                    nc.vector.tensor_scalar_add(dst[c][:, ns], ps,
                                                b_sb[:, c:c + 1])

        def v_piece(n):
            nsl = slice(n * 128, (n + 1) * 128)
            for i in range(2):
                cs = slice(i * HALF, (i + 1) * HALF)
                ps = PS_S.tile([128, HALF], F32)
                nc.tensor.matmul(ps, xt_all[0][:, nsl], wv_sb[:, 0, cs],
                                 start=True, stop=False)
                nc.tensor.matmul(ps, xt_all[1][:, nsl], wv_sb[:, 1, cs],
                                 start=False, stop=True)
                nc.vector.tensor_add(v_sb[n][:, cs], ps, wvbb_sb[:, cs])

        # qt/kt c0 first so block 0's S matmuls can start immediately
        qkt_piece("wq", wqb_sb, qt_sb, 0, True)
        qkt_piece("wk", wkb_sb, kt_sb, 0, False)

        # remaining pieces woven into blocks: V into block 0 (needed by the
        # first ones/pv in block 1), qt/kt chunk c before block 2c
        pieces = [lambda n=n: v_piece(n) for n in range(8)]
        for c in range(1, 8):
            pieces.append(lambda c=c: qkt_piece("wq", wqb_sb, qt_sb, c, True))
            pieces.append(lambda c=c: qkt_piece("wk", wkb_sb, kt_sb, c, False))
        piece_quota = {0: 8, 1: 2, 2: 2, 3: 2, 4: 2, 5: 2, 6: 2, 7: 2}

        # ---- pipelined block loop: block t = (head h, n-half i) ----
        at_t = {}     # (t, m) -> at tile
        pv_t = {}     # t -> pv psum tile
        rs_t = {}     # t -> rowsum psum tile
        rc_t = {}     # t -> reciprocal rowsum [1, HALF]
        bcp_t = {}    # t -> PE-broadcast recip psum tile
        oh_t = {}     # t -> normalized head-output tile

        def s_exp_mul(t, m):
            h, i = divmod(t, 2)
            ns = slice(i * HALF, (i + 1) * HALF)
            ms = slice(m * 128, (m + 1) * 128)
            ps = PS_S.tile([128, HALF], F32)
            nc.tensor.matmul(ps, kt_sb[h][:, ms], qt_sb[h][:, ns],
                             start=True, stop=True)
            at = AT.tile([128, HALF], F32R)
            nc.scalar.activation(at, ps, func=EXP)
            eng = nc.gpsimd if m < POOL_MULS else nc.vector
            eng.tensor_mul(at, at, eb_view(m)[:, ns])
            at_t[(t, m)] = at

        def ones_pv(t, m):
            h, _ = divmod(t, 2)
            hs = slice(h * 128, (h + 1) * 128)
            if m == 0:
                rs_t[t] = PS_RS.tile([1, HALF], F32, tag="rs", name=f"rs{t}")
                pv_t[t] = PS_PV.tile([128, HALF], F32, tag="pv", name=f"pv{t}")
            at = at_t.pop((t, m))
            nc.tensor.matmul(rs_t[t], ones, at, start=(m == 0), stop=(m == 7))
            nc.tensor.matmul(pv_t[t], v_sb[m][:, hs], at,
                             start=(m == 0), stop=(m == 7))

        from concourse.dve_ops import (
            RECIP_APPROX_FAST_CONSTS,
            RECIPROCAL_APPROX_FAST,
        )

        def recip(t):
            # softmax denominators: 1/rowsum, approx (~18 good bits), f32r
            # out so the broadcast matmul can consume it directly
            rc = RC.tile([1, HALF], F32R, tag="rc", name=f"rc{t}")
            cc = RECIP_APPROX_FAST_CONSTS
            nc.vector._custom_dve(RECIPROCAL_APPROX_FAST, out=rc,
                                  in0=rs_t.pop(t), s0=cc["s0"], s1=cc["s1"],
                                  imm2=cc["imm2"])
            rc_t[t] = rc

        def bcp_mm(t):
            # partition-broadcast recip via contraction-1 matmul (no DMA)
            bcp = PS_BC.tile([128, HALF], F32, tag="bcp", name=f"bcp{t}")
            nc.tensor.matmul(bcp, ones_row, rc_t.pop(t),
                             start=True, stop=True)
            bcs = BCS.tile([128, HALF], F32, tag="bcs", name=f"bcs{t}")
            nc.scalar.activation(bcs, bcp, func=IDENT)
            bcp_t[t] = bcs

        def oh_mul(t):
            oh = OH.tile([128, HALF], F32R, tag="oh", name=f"oh{t}")
            nc.vector.tensor_mul(oh, pv_t.pop(t), bcp_t.pop(t))
            oh_t[t] = oh

        def proj_acc(t):
            h, i = divmod(t, 2)
            ns = slice(i * HALF, (i + 1) * HALF)
            pj = PS_PJ.tile([128, HALF], F32, tag="pj", name=f"pj{t}")
            nc.tensor.matmul(pj, pw_view(h), oh_t.pop(t),
                             start=True, stop=True)
            if h == 0:
                nc.vector.tensor_copy(yacc[:, ns], pj)
            else:
                nc.vector.tensor_add(yacc[:, ns], yacc[:, ns], pj)

        def finalize(i):
            ns = slice(i * HALF, (i + 1) * HALF)
            nc.scalar.activation(yt_sb[:, ns], yacc[:, ns], func=IDENT,
                                 bias=pb_sb)
            nc.sync.dma_start(out=yT[:, ns], in_=yt_sb[:, ns])

        pi = 0
        for t in range(NBLK + 2):
            quota = piece_quota.get(t, 0)
            for m in range(8):
                if t < NBLK:
                    s_exp_mul(t, m)
                if 1 <= t <= NBLK:
                    ones_pv(t - 1, m)
                if m == 2 and 2 <= t:
                    bcp_mm(t - 2)    # recip(t-2) done by now; 213ns on PE
                if m == 4 and 2 <= t:
                    oh_mul(t - 2)    # bcp just above; frees pv(t-2)
                if quota and m % (8 // quota) == (8 // quota) - 1:
                    pieces[pi](); pi += 1
            if 1 <= t <= NBLK:
                recip(t - 1)         # rs(t-1) just stopped
            if 2 <= t:
                proj_acc(t - 2)      # PE reaches this after the block's work
                if t - 2 >= NBLK - 2:
                    finalize((t - 2) % 2)
        assert pi == len(pieces)


_CACHE = {}


def _prep_inputs(x, B_bias, wq_w, wq_b, wk_w, wk_b, wv_w, wv_b, proj_w, proj_b):
    s = 1.0 / math.sqrt(DH)
    f = np.float32

    def d2(w):  # [256, C] -> [2, 128, C]
        return np.asarray(w, f).reshape(2, 128, -1)

    wq3 = d2(np.asarray(wq_w) * s)
    wk3 = d2(wk_w)
    wv3 = d2(wv_w)
    # wqk0: [p, w, a, c0] packed
    wqk0 = np.stack([wq3[:, :, :128], wk3[:, :, :128]], 0)  # [w, a, p, 128]
    wqk0 = np.ascontiguousarray(wqk0.transpose(2, 0, 1, 3).reshape(128, -1))
    # wbig per d: [wq_r 896 | wk_r 896 | wv 1024 | pw-half 512]
    pwf = np.asarray(proj_w, f).reshape(2, 512, DH)  # head-halves 0-3 / 4-7
    rows = []
    for d in range(2):
        pw_tail = pwf[d].reshape(4, 128, DH)
        pw_part = pw_tail.transpose(1, 0, 2).reshape(128, 512)
        rows.append(np.concatenate(
            [wq3[d, :, 128:], wk3[d, :, 128:], pw_part], axis=1))
    wbig = np.ascontiguousarray(np.stack(rows, 1).reshape(128, -1))
    wv_p = np.ascontiguousarray(wv3.transpose(1, 0, 2).reshape(128, -1))
    # biases: [wqb 8 | wkb 8 | wvbb 1024 | pb 1]
    wqb_t = (np.asarray(wq_b, f) * s).reshape(8, 128).T
    wkb_t = np.asarray(wk_b, f).reshape(8, 128).T
    wvbb = np.broadcast_to(np.asarray(wv_b, f), (128, C))
    pb_t = np.asarray(proj_b, f).reshape(128, 1)
    bias_all = np.ascontiguousarray(
        np.concatenate([wqb_t, wkb_t, wvbb, pb_t], axis=1))
    # eb: exp(B)^T chunks packed [p, (m n)]
    ebh = np.exp(np.asarray(B_bias, np.float32).T).reshape(8, 128, N)
    eb_all = np.ascontiguousarray(
        ebh.transpose(1, 0, 2).reshape(128, 8 * N))
    xTh = np.asarray(x, f).transpose(0, 2, 1).reshape(8, 2, 128, N)
    shared = dict(wqk0=wqk0, wbig=wbig, wv=wv_p, biases=bias_all, eb=eb_all)
    return [dict(shared, xt=np.ascontiguousarray(
        xTh[b].transpose(1, 0, 2).reshape(128, 2 * N))) for b in range(NCORES)]


def kernel(**inputs):
    from concourse.bass_utils import run_bass_kernel_spmd

    if "nc" not in _CACHE:
        _CACHE["nc"] = build_nc()
    nc = _CACHE["nc"]
    in_maps = _prep_inputs(**inputs)
    res = run_bass_kernel_spmd(nc, in_maps, core_ids=list(range(NCORES)))
    out = np.stack([np.asarray(res.results[b]["yT"]).T for b in range(NCORES)])
    return np.ascontiguousarray(out.astype(np.float32))
